# revision 29
# baseline (speedup 1.0000x reference)
import numpy as np

# nn_MultiHeadedAttention: B=4, S=2048, D_MODEL=1024, H=16, D_K=64, fp32.
# Sharding: 8 cores = 4 batches x 2 head-groups (8 heads each).
# V2 design:
#  - bf16 staging everywhere, fp32 PSUM accumulation (rel err ~7e-3).
#  - scores: two heads in PE row groups (64-contraction row tiling) run
#    concurrently -> 2x score throughput.
#  - software pipeline over 16 units (p, qc): slot u emits scores(u),
#    PV(u-1), reciprocal(u-1), normalize(u-2); ACT exp is the pacing
#    engine and starts ~35us in (right after K + Q(qc0) projections).
#  - V projection interleaved into slot 0, Q(qc+1) at each round end,
#    output projection per finished qc round.
#  - softmax denominator via appended ones-column (PV row 64),
#    reciprocal_approx_fast, PE-broadcast, DVE multiply.

B, S, D, H, DK = 4, 2048, 1024, 16, 64
NCORES = 8
DG = 512  # dims per head-group (8 heads x 64)

_NC_CACHE = {}
LAST_EXEC_NS = None


def _build_nc():
    import concourse.bacc as bacc
    import concourse.tile as tile
    from concourse import mybir

    F32R = mybir.dt.float32r
    F32 = mybir.dt.float32
    BF16 = mybir.dt.bfloat16
    EXP = mybir.ActivationFunctionType.Exp

    nc = bacc.Bacc(None, target_bir_lowering=False, debug=True)

    xqT = nc.dram_tensor("xqT", [D, S], BF16, kind="ExternalInput")
    xkT = nc.dram_tensor("xkT", [D, S], BF16, kind="ExternalInput")
    xvT = nc.dram_tensor("xvT", [D, S], BF16, kind="ExternalInput")
    wqT = nc.dram_tensor("wqT", [D, DG], BF16, kind="ExternalInput")
    wkT = nc.dram_tensor("wkT", [D, DG], BF16, kind="ExternalInput")
    wvT = nc.dram_tensor("wvT", [D, DG], BF16, kind="ExternalInput")
    woT = nc.dram_tensor("woT", [DG, D], BF16, kind="ExternalInput")
    bqc = nc.dram_tensor("bqc", [128, 4], F32, kind="ExternalInput")
    bkc = nc.dram_tensor("bkc", [128, 4], F32, kind="ExternalInput")
    bvr = nc.dram_tensor("bvr", [DG], F32, kind="ExternalInput")
    y_d = nc.dram_tensor("y", [S, D], F32R, kind="ExternalOutput")

    import concourse.bass as bass

    with (
        tile.TileContext(nc) as tc,
        nc.allow_low_precision(reason="bf16 staging, fp32 accumulation"),
        tc.tile_pool(name="persist", bufs=1) as persist,
    ):
        QT = [persist.tile([128, S], BF16, name=f"QT{p}") for p in range(4)]
        KT = [persist.tile([128, S], BF16, name=f"KT{p}") for p in range(4)]
        AT = [persist.tile([128, S], BF16, name=f"AT{p}") for p in range(4)]
        vones = [persist.tile([128, 8, 65], BF16, name=f"vones{s}") for s in range(16)]
        wo = persist.tile([128, 4, D], BF16, name="wo")
        bq_sb = persist.tile([128, 4], F32, name="bq_sb")
        bk_sb = persist.tile([128, 4], F32, name="bk_sb")
        bv_sb = persist.tile([128, DG], F32, name="bv_sb")

        nc.gpsimd.dma_start(bq_sb[:], bqc[:])
        nc.gpsimd.dma_start(bk_sb[:], bkc[:])
        bv_ap = bvr[:]
        bv_bcast = bass.AP(tensor=bv_ap.tensor, offset=bv_ap.offset, ap=[[0, 128], *bv_ap.ap])
        nc.gpsimd.dma_start(bv_sb[:], bv_bcast)
        for s in range(16):
            nc.vector.memset(vones[s][:, :, 64:65], 1.0)

        warena_cm = tc.tile_pool(name="warena", bufs=2)
        warena = warena_cm.__enter__()
        stage_cm = tc.tile_pool(name="stage", bufs=3)
        stage = stage_cm.__enter__()
        ps_st_cm = tc.tile_pool(name="ps_st", bufs=2, space="PSUM")
        ps_st = ps_st_cm.__enter__()
        ps_pv_cm = tc.tile_pool(name="ps_pv", bufs=2, space="PSUM")
        ps_pv = ps_pv_cm.__enter__()
        ech_cm = tc.tile_pool(name="ech", bufs=32)
        ech_pool = ech_cm.__enter__()
        rec_cm = tc.tile_pool(name="recp", bufs=2)
        rec_pool = rec_cm.__enter__()
        bcs_cm = tc.tile_pool(name="bcsp", bufs=2)
        bcs_pool = bcs_cm.__enter__()
        oy_cm = tc.tile_pool(name="oy", bufs=2)
        oy = oy_cm.__enter__()

        def load_w(w_d, eng):
            wt = warena.tile([128, 8, DG], BF16, name="wt")
            for i in range(8):
                eng.dma_start(wt[:, i, :], w_d[i * 128 : (i + 1) * 128, :])
            return wt

        wk = load_w(wkT, nc.scalar)

        # ---- K projection: all p, all qc ----
        for qc in range(4):
            qs = slice(qc * 512, (qc + 1) * 512)
            xs = stage.tile([128, 8, 512], BF16, name="xs", bufs=2)
            for i in range(8):
                eng = nc.sync if i % 2 == 0 else nc.gpsimd
                eng.dma_start(xs[:, i, :], xkT[i * 128 : (i + 1) * 128, qs])
            for p in range(4):
                pp = ps_st.tile([128, 2, 512], F32, name="st")
                for i in range(8):
                    nc.tensor.matmul(
                        pp[:, 0, :],
                        wk[:, i, p * 128 : (p + 1) * 128],
                        xs[:, i, :],
                        start=(i == 0),
                        stop=(i == 7),
                    )
                nc.vector.tensor_scalar_add(KT[p][:, qs], pp[:, 0, :], bk_sb[:, p : p + 1])

        # off-critical-path weights load after K projection; wq reuses
        # wk's arena buffer (WAR on K-proj reads, emitted above)
        wv = load_w(wvT, nc.scalar)
        for p in range(4):
            nc.sync.dma_start(wo[:, p, :], woT[p * 128 : (p + 1) * 128, :])
        wq = load_w(wqT, nc.sync)

        def q_proj(qc):
            qs = slice(qc * 512, (qc + 1) * 512)
            xs = stage.tile([128, 8, 512], BF16, name="xs", bufs=2)
            for i in range(8):
                eng = nc.sync if i % 2 == 0 else nc.gpsimd
                eng.dma_start(xs[:, i, :], xqT[i * 128 : (i + 1) * 128, qs])
            for p in range(4):
                pp = ps_st.tile([128, 2, 512], F32, name="st")
                for i in range(8):
                    nc.tensor.matmul(
                        pp[:, 0, :],
                        wq[:, i, p * 128 : (p + 1) * 128],
                        xs[:, i, :],
                        start=(i == 0),
                        stop=(i == 7),
                    )
                nc.vector.tensor_scalar_add(QT[p][:, qs], pp[:, 0, :], bq_sb[:, p : p + 1])

        q_proj(0)

        def v_chunk(sb):
            ss = slice(sb * 128, (sb + 1) * 128)
            xv = stage.tile([128, 8, 128], BF16, name="xv")
            for i in range(8):
                eng = nc.sync if i % 2 == 0 else nc.gpsimd
                eng.dma_start(xv[:, i, :], xvT[i * 128 : (i + 1) * 128, ss])
            vp = ps_pv.tile([128, 2, 512], F32, name="pvt")
            for i in range(8):
                nc.tensor.matmul(
                    vp[:, 0, :], xv[:, i, :], wv[:, i, :], start=(i == 0), stop=(i == 7)
                )
            nc.vector.tensor_add(
                vones[sb][:, :, 0:64],
                vp[:, 0, :].rearrange("p (h d) -> p h d", h=8),
                bv_sb[:].rearrange("p (h d) -> p h d", h=8),
            )

        def out_proj(qc):
            for sb in range(qc * 4, qc * 4 + 4):
                ss = slice(sb * 128, (sb + 1) * 128)
                ys = oy.tile([128, 2, 512], F32R, name="ys")
                yp = ps_st.tile([128, 2, 512], F32, name="st")
                for oc in range(2):
                    for p in range(4):
                        nc.tensor.matmul(
                            yp[:, oc, :],
                            AT[p][:, ss],
                            wo[:, p, oc * 512 : (oc + 1) * 512],
                            start=(p == 0),
                            stop=(p == 3),
                        )
                nc.vector.tensor_copy(ys[:], yp[:])
                eng = (nc.gpsimd, nc.sync)[sb % 2]
                eng.dma_start(y_d[ss, :], ys[:])

        # ---- attention software pipeline over 16 units ----
        units = [(p, qc) for qc in range(4) for p in range(4)]
        ech_of = {}
        pvt_of = {}
        rec_of = {}

        def sc_pair(u, kb):
            p, qc = units[u]
            qs = slice(qc * 512, (qc + 1) * 512)
            tiles = ech_of.setdefault(u, [])
            st = ps_st.tile([128, 2, 512], F32, name="st")
            ech = ech_pool.tile([128, 2, 512], BF16, name="ech")
            tiles.append(ech)
            for h in range(2):
                nc.tensor.matmul(
                    st[:, h, :],
                    KT[p][h * 64 : (h + 1) * 64, kb * 128 : (kb + 1) * 128],
                    QT[p][h * 64 : (h + 1) * 64, qs],
                    start=True,
                    stop=True,
                )
            nc.scalar.activation(out=ech[:], in_=st[:], func=EXP, scale=0.125)

        def pv8(u, h, half):
            # 8 accumulating PV matmuls: head h, k-blocks half*8..half*8+7
            p, qc = units[u]
            if h == 0 and half == 0:
                pvt_of[u] = ps_pv.tile([128, 2, 512], F32, name="pvt")
            pvt = pvt_of[u]
            tiles = ech_of[u]
            hidx = p * 2 + h
            for kbj in range(8):
                kb = half * 8 + kbj
                nc.tensor.matmul(
                    pvt[0:65, h, :],
                    vones[kb][:, hidx, :],
                    tiles[kb][:, h, :],
                    start=(half == 0 and kbj == 0),
                    stop=(half == 1 and kbj == 7),
                )

        def recip(u):
            pvt = pvt_of[u]
            rec = rec_pool.tile([128, 2, 512], F32, name="rec")
            rec_of[u] = rec
            for h in range(2):
                nc.vector.reciprocal(rec[0:1, h, :], pvt[64:65, h, :])

        def norm(u):
            p, qc = units[u]
            qs = slice(qc * 512, (qc + 1) * 512)
            pvt, rec = pvt_of.pop(u), rec_of.pop(u)
            del ech_of[u]
            bcs = bcs_pool.tile([64, 2, 512], F32, name="bcs")
            nc.gpsimd.partition_broadcast(bcs[:], rec[0:1, :, :])
            for h in range(2):
                hb = h * 64
                nc.vector.tensor_mul(
                    AT[p][hb : hb + 64, qs], pvt[0:64, h, :], bcs[:, h, :]
                )

        # q_proj split: per-round xq load + per-slot single-p MM chain
        xs_of_round = {}

        def q_load(qc):
            qs = slice(qc * 512, (qc + 1) * 512)
            xs = stage.tile([128, 8, 512], BF16, name="xs", bufs=2)
            xs_of_round[qc] = xs
            for i in range(8):
                eng = nc.sync if i % 2 == 0 else nc.gpsimd
                eng.dma_start(xs[:, i, :], xqT[i * 128 : (i + 1) * 128, qs])

        def q_chain(qc, p, half):
            # self-contained half: full contraction into 256 output cols
            q0 = qc * 512 + half * 256
            qs = slice(q0, q0 + 256)
            xs = xs_of_round[qc]
            pp = ps_st.tile([128, 2, 512], F32, name="st")
            for i in range(8):
                nc.tensor.matmul(
                    pp[:, 0, 0:256],
                    wq[:, i, p * 128 : (p + 1) * 128],
                    xs[:, i, half * 256 : half * 256 + 256],
                    start=(i == 0),
                    stop=(i == 7),
                )
            nc.vector.tensor_scalar_add(
                QT[p][:, qs], pp[:, 0, 0:256], bq_sb[:, p : p + 1]
            )

        ys_of = {}

        def out_sb(sb, oc):
            # self-contained: one oc = 4 accumulating matmuls + evac
            ss = slice(sb * 128, (sb + 1) * 128)
            if oc == 0:
                ys_of[sb] = oy.tile([128, 2, 512], F32R, name="ys")
            yp = ps_st.tile([128, 2, 512], F32, name="st")
            for p in range(4):
                nc.tensor.matmul(
                    yp[:, 0, :],
                    AT[p][:, ss],
                    wo[:, p, oc * 512 : (oc + 1) * 512],
                    start=(p == 0),
                    stop=(p == 3),
                )
            ys = ys_of[sb]
            nc.vector.tensor_copy(ys[:, oc, :], yp[:, 0, :])
            if oc == 1:
                del ys_of[sb]
                eng = (nc.gpsimd, nc.sync)[sb % 2]
                eng.dma_start(y_d[ss, :], ys[:])

        for u in range(16):
            p, qc = units[u]
            if p == 0 and qc < 3:
                q_load(qc + 1)
            # one block of non-score PE work per rep of 2 score pairs;
            # blocks are kept <= ~2us so the 2-deep st ring never lets
            # ACT run dry, and PE stays dense (HAM stays at K=8/8)
            blocks = []
            if u == 0:
                blocks += [lambda sb=sb: v_chunk(sb) for sb in range(8)]
                blocks += [lambda: q_chain(1, 0, 0), lambda: q_chain(1, 0, 1)]
            elif u == 1:
                blocks += [lambda sb=sb: v_chunk(sb) for sb in range(8, 16)]
                blocks += [
                    lambda: pv8(0, 0, 0),
                    lambda: pv8(0, 1, 0),
                    lambda: pv8(0, 0, 1),
                    lambda: (pv8(0, 1, 1), recip(0)),
                    lambda: q_chain(1, 1, 0),
                    lambda: q_chain(1, 1, 1),
                ]
            else:
                blocks += [
                    lambda: pv8(u - 1, 0, 0),
                    lambda: (pv8(u - 1, 1, 0), norm(u - 2)),
                    lambda: pv8(u - 1, 0, 1),
                    lambda: (pv8(u - 1, 1, 1), recip(u - 1)),
                ]
                if qc < 3:
                    blocks.append(lambda: q_chain(qc + 1, p, 0))
                    blocks.append(lambda: q_chain(qc + 1, p, 1))
                if u >= 5:
                    blocks.append(lambda: out_sb(u - 5, 0))
                    blocks.append(lambda: out_sb(u - 5, 1))
            bi = 0
            for kb in range(0, 16, 2):
                sc_pair(u, kb)
                sc_pair(u, kb + 1)
                if bi < len(blocks):
                    blocks[bi]()
                    bi += 1
            while bi < len(blocks):
                blocks[bi]()
                bi += 1
        # tail: unit 15
        pv8(15, 0, 0)
        pv8(15, 1, 0)
        norm(14)
        pv8(15, 0, 1)
        pv8(15, 1, 1)
        recip(15)
        norm(15)
        for sb in range(11, 16):
            out_sb(sb, 0)
            out_sb(sb, 1)

        for cm in (oy_cm, bcs_cm, rec_cm, ech_cm, ps_pv_cm, ps_st_cm, stage_cm, warena_cm):
            cm.__exit__(None, None, None)

    nc.compile()
    return nc


def _get_nc():
    if "nc" not in _NC_CACHE:
        _NC_CACHE["nc"] = _build_nc()
    return _NC_CACHE["nc"]


def kernel(**inputs):
    import ml_dtypes
    from concourse import bass_utils

    BF = ml_dtypes.bfloat16
    q, k, v = inputs["query"], inputs["key"], inputs["value"]
    Wq, Wk, Wv, Wo = inputs["Wq"], inputs["Wk"], inputs["Wv"], inputs["Wo"]
    bq, bk, bv, bo = inputs["bq"], inputs["bk"], inputs["bv"], inputs["bo"]

    nc = _get_nc()
    in_maps = []
    for c in range(NCORES):
        b, hg = divmod(c, 2)
        r0 = hg * DG
        rs = slice(r0, r0 + DG)
        in_maps.append(
            {
                "xqT": np.ascontiguousarray(q[b].T.astype(BF)),
                "xkT": np.ascontiguousarray(k[b].T.astype(BF)),
                "xvT": np.ascontiguousarray(v[b].T.astype(BF)),
                "wqT": np.ascontiguousarray(Wq[rs, :].T.astype(BF)),
                "wkT": np.ascontiguousarray(Wk[rs, :].T.astype(BF)),
                "wvT": np.ascontiguousarray(Wv[rs, :].T.astype(BF)),
                "woT": np.ascontiguousarray(Wo[:, rs].T.astype(BF)),
                "bqc": np.ascontiguousarray(bq[rs].reshape(4, 128).T),
                "bkc": np.ascontiguousarray(bk[rs].reshape(4, 128).T),
                "bvr": np.ascontiguousarray(bv[rs]),
            }
        )
    import os

    trace = bool(os.environ.get("KERNEL_TRACE"))
    res = bass_utils.run_bass_kernel_spmd(
        nc, in_maps, core_ids=list(range(NCORES)), trace=trace
    )
    global LAST_EXEC_NS
    LAST_EXEC_NS = res.exec_time_ns
    out = np.empty((B, S, D), np.float32)
    for b in range(B):
        out[b] = res.results[2 * b]["y"] + res.results[2 * b + 1]["y"] + bo[None, :]
    return out


# revision 32
# speedup vs baseline: 1.0304x; 1.0304x over previous
import numpy as np

# nn_MultiHeadedAttention: B=4, S=2048, D_MODEL=1024, H=16, D_K=64, fp32.
# Sharding: 8 cores = 4 batches x 2 head-groups (8 heads each).
# V2 design:
#  - bf16 staging everywhere, fp32 PSUM accumulation (rel err ~7e-3).
#  - scores: two heads in PE row groups (64-contraction row tiling) run
#    concurrently -> 2x score throughput.
#  - software pipeline over 16 units (p, qc): slot u emits scores(u),
#    PV(u-1), reciprocal(u-1), normalize(u-2); ACT exp is the pacing
#    engine and starts ~35us in (right after K + Q(qc0) projections).
#  - V projection interleaved into slot 0, Q(qc+1) at each round end,
#    output projection per finished qc round.
#  - softmax denominator via appended ones-column (PV row 64),
#    reciprocal_approx_fast, PE-broadcast, DVE multiply.

B, S, D, H, DK = 4, 2048, 1024, 16, 64
NCORES = 8
DG = 512  # dims per head-group (8 heads x 64)

_NC_CACHE = {}
LAST_EXEC_NS = None


def _build_nc():
    import concourse.bacc as bacc
    import concourse.tile as tile
    from concourse import mybir

    F32R = mybir.dt.float32r
    F32 = mybir.dt.float32
    BF16 = mybir.dt.bfloat16
    EXP = mybir.ActivationFunctionType.Exp

    nc = bacc.Bacc(None, target_bir_lowering=False, debug=True)

    xqT = nc.dram_tensor("xqT", [D, S], BF16, kind="ExternalInput")
    xkT = nc.dram_tensor("xkT", [D, S], BF16, kind="ExternalInput")
    xvT = nc.dram_tensor("xvT", [D, S], BF16, kind="ExternalInput")
    wqT = nc.dram_tensor("wqT", [D, DG], BF16, kind="ExternalInput")
    wkT = nc.dram_tensor("wkT", [D, DG], BF16, kind="ExternalInput")
    wvT = nc.dram_tensor("wvT", [D, DG], BF16, kind="ExternalInput")
    woT = nc.dram_tensor("woT", [DG, D], BF16, kind="ExternalInput")
    bqc = nc.dram_tensor("bqc", [128, 4], F32, kind="ExternalInput")
    bkc = nc.dram_tensor("bkc", [128, 4], F32, kind="ExternalInput")
    bvr = nc.dram_tensor("bvr", [DG], F32, kind="ExternalInput")
    y_d = nc.dram_tensor("y", [S, D], F32R, kind="ExternalOutput")

    import concourse.bass as bass

    with (
        tile.TileContext(nc) as tc,
        nc.allow_low_precision(reason="bf16 staging, fp32 accumulation"),
        tc.tile_pool(name="persist", bufs=1) as persist,
    ):
        QT = [persist.tile([128, S], BF16, name=f"QT{p}") for p in range(4)]
        KT = [persist.tile([128, S], BF16, name=f"KT{p}") for p in range(4)]
        AT = [persist.tile([128, S], BF16, name=f"AT{p}") for p in range(4)]
        vones = [persist.tile([128, 8, 65], BF16, name=f"vones{s}") for s in range(16)]
        wo = persist.tile([128, 4, D], BF16, name="wo")
        bq_sb = persist.tile([128, 4], F32, name="bq_sb")
        bk_sb = persist.tile([128, 4], F32, name="bk_sb")
        bv_sb = persist.tile([128, DG], F32, name="bv_sb")

        nc.gpsimd.dma_start(bq_sb[:], bqc[:])
        nc.gpsimd.dma_start(bk_sb[:], bkc[:])
        bv_ap = bvr[:]
        bv_bcast = bass.AP(tensor=bv_ap.tensor, offset=bv_ap.offset, ap=[[0, 128], *bv_ap.ap])
        nc.gpsimd.dma_start(bv_sb[:], bv_bcast)
        for s in range(16):
            nc.vector.memset(vones[s][:, :, 64:65], 1.0)

        warena_cm = tc.tile_pool(name="warena", bufs=2)
        warena = warena_cm.__enter__()
        stage_cm = tc.tile_pool(name="stage", bufs=3)
        stage = stage_cm.__enter__()
        ps_st_cm = tc.tile_pool(name="ps_st", bufs=2, space="PSUM")
        ps_st = ps_st_cm.__enter__()
        ps_pv_cm = tc.tile_pool(name="ps_pv", bufs=2, space="PSUM")
        ps_pv = ps_pv_cm.__enter__()
        ech_cm = tc.tile_pool(name="ech", bufs=32)
        ech_pool = ech_cm.__enter__()
        rec_cm = tc.tile_pool(name="recp", bufs=2)
        rec_pool = rec_cm.__enter__()
        bcs_cm = tc.tile_pool(name="bcsp", bufs=2)
        bcs_pool = bcs_cm.__enter__()
        oy_cm = tc.tile_pool(name="oy", bufs=2)
        oy = oy_cm.__enter__()

        def load_w(w_d, eng):
            wt = warena.tile([128, 8, DG], BF16, name="wt")
            for i in range(8):
                eng.dma_start(wt[:, i, :], w_d[i * 128 : (i + 1) * 128, :])
            return wt

        wk = load_w(wkT, nc.scalar)

        # ---- K projection: all p, all qc ----
        for qc in range(4):
            qs = slice(qc * 512, (qc + 1) * 512)
            xs = stage.tile([128, 8, 512], BF16, name="xs", bufs=2)
            for i in range(8):
                eng = nc.sync if i % 2 == 0 else nc.gpsimd
                eng.dma_start(xs[:, i, :], xkT[i * 128 : (i + 1) * 128, qs])
            for p in range(4):
                pp = ps_st.tile([128, 2, 512], F32, name="st")
                for i in range(8):
                    nc.tensor.matmul(
                        pp[:, 0, :],
                        wk[:, i, p * 128 : (p + 1) * 128],
                        xs[:, i, :],
                        start=(i == 0),
                        stop=(i == 7),
                    )
                nc.vector.tensor_scalar_add(KT[p][:, qs], pp[:, 0, :], bk_sb[:, p : p + 1])

        # off-critical-path weights load after K projection; wq reuses
        # wk's arena buffer (WAR on K-proj reads, emitted above)
        wv = load_w(wvT, nc.scalar)
        for p in range(4):
            nc.sync.dma_start(wo[:, p, :], woT[p * 128 : (p + 1) * 128, :])
        wq = load_w(wqT, nc.sync)

        def q_proj(qc):
            qs = slice(qc * 512, (qc + 1) * 512)
            xs = stage.tile([128, 8, 512], BF16, name="xs", bufs=2)
            for i in range(8):
                eng = nc.sync if i % 2 == 0 else nc.gpsimd
                eng.dma_start(xs[:, i, :], xqT[i * 128 : (i + 1) * 128, qs])
            for p in range(4):
                pp = ps_st.tile([128, 2, 512], F32, name="st")
                for i in range(8):
                    nc.tensor.matmul(
                        pp[:, 0, :],
                        wq[:, i, p * 128 : (p + 1) * 128],
                        xs[:, i, :],
                        start=(i == 0),
                        stop=(i == 7),
                    )
                nc.vector.tensor_scalar_add(QT[p][:, qs], pp[:, 0, :], bq_sb[:, p : p + 1])

        q_proj(0)

        def v_chunk(sb):
            ss = slice(sb * 128, (sb + 1) * 128)
            xv = stage.tile([128, 8, 128], BF16, name="xv")
            for i in range(8):
                eng = nc.sync if i % 2 == 0 else nc.gpsimd
                eng.dma_start(xv[:, i, :], xvT[i * 128 : (i + 1) * 128, ss])
            vp = ps_pv.tile([128, 2, 512], F32, name="pvt")
            for i in range(8):
                nc.tensor.matmul(
                    vp[:, 0, :], xv[:, i, :], wv[:, i, :], start=(i == 0), stop=(i == 7)
                )
            nc.vector.tensor_add(
                vones[sb][:, :, 0:64],
                vp[:, 0, :].rearrange("p (h d) -> p h d", h=8),
                bv_sb[:].rearrange("p (h d) -> p h d", h=8),
            )

        def out_proj(qc):
            for sb in range(qc * 4, qc * 4 + 4):
                ss = slice(sb * 128, (sb + 1) * 128)
                ys = oy.tile([128, 2, 512], F32R, name="ys")
                yp = ps_st.tile([128, 2, 512], F32, name="st")
                for oc in range(2):
                    for p in range(4):
                        nc.tensor.matmul(
                            yp[:, oc, :],
                            AT[p][:, ss],
                            wo[:, p, oc * 512 : (oc + 1) * 512],
                            start=(p == 0),
                            stop=(p == 3),
                        )
                nc.vector.tensor_copy(ys[:], yp[:])
                eng = (nc.gpsimd, nc.sync)[sb % 2]
                eng.dma_start(y_d[ss, :], ys[:])

        # ---- attention software pipeline over 16 units ----
        units = [(p, qc) for qc in range(4) for p in range(4)]
        ech_of = {}
        pvt_of = {}
        rec_of = {}

        def sc_pair(u, kb):
            p, qc = units[u]
            qs = slice(qc * 512, (qc + 1) * 512)
            tiles = ech_of.setdefault(u, [])
            st = ps_st.tile([128, 2, 512], F32, name="st")
            ech = ech_pool.tile([128, 2, 512], BF16, name="ech")
            tiles.append(ech)
            for h in range(2):
                nc.tensor.matmul(
                    st[:, h, :],
                    KT[p][h * 64 : (h + 1) * 64, kb * 128 : (kb + 1) * 128],
                    QT[p][h * 64 : (h + 1) * 64, qs],
                    start=True,
                    stop=True,
                )
            nc.scalar.activation(out=ech[:], in_=st[:], func=EXP, scale=0.125)

        def pv8(u, h, half):
            # 8 accumulating PV matmuls: head h, k-blocks half*8..half*8+7
            p, qc = units[u]
            if h == 0 and half == 0:
                pvt_of[u] = ps_pv.tile([128, 2, 512], F32, name="pvt")
            pvt = pvt_of[u]
            tiles = ech_of[u]
            hidx = p * 2 + h
            for kbj in range(8):
                kb = half * 8 + kbj
                nc.tensor.matmul(
                    pvt[0:65, h, :],
                    vones[kb][:, hidx, :],
                    tiles[kb][:, h, :],
                    start=(half == 0 and kbj == 0),
                    stop=(half == 1 and kbj == 7),
                )

        bcs_of = {}

        def recip(u):
            pvt = pvt_of[u]
            rec = rec_pool.tile([128, 2, 512], F32, name="rec")
            rec_of[u] = rec
            for h in range(2):
                nc.vector.reciprocal(rec[0:1, h, :], pvt[64:65, h, :])
            # broadcast immediately: gives GpSimd a full slot of slack
            # before norm's multiplies need bcs
            bcs = bcs_pool.tile([64, 2, 512], F32, name="bcs")
            bcs_of[u] = bcs
            nc.gpsimd.partition_broadcast(bcs[:], rec[0:1, :, :])

        def norm(u):
            p, qc = units[u]
            qs = slice(qc * 512, (qc + 1) * 512)
            pvt = pvt_of.pop(u)
            rec_of.pop(u)
            bcs = bcs_of.pop(u)
            del ech_of[u]
            for h in range(2):
                hb = h * 64
                nc.vector.tensor_mul(
                    AT[p][hb : hb + 64, qs], pvt[0:64, h, :], bcs[:, h, :]
                )

        # q_proj split: per-round xq load + per-slot single-p MM chain
        xs_of_round = {}

        def q_load(qc):
            qs = slice(qc * 512, (qc + 1) * 512)
            xs = stage.tile([128, 8, 512], BF16, name="xs", bufs=2)
            xs_of_round[qc] = xs
            for i in range(8):
                eng = nc.sync if i % 2 == 0 else nc.gpsimd
                eng.dma_start(xs[:, i, :], xqT[i * 128 : (i + 1) * 128, qs])

        def q_chain(qc, p, half):
            # self-contained half: full contraction into 256 output cols
            q0 = qc * 512 + half * 256
            qs = slice(q0, q0 + 256)
            xs = xs_of_round[qc]
            pp = ps_st.tile([128, 2, 512], F32, name="st")
            for i in range(8):
                nc.tensor.matmul(
                    pp[:, 0, 0:256],
                    wq[:, i, p * 128 : (p + 1) * 128],
                    xs[:, i, half * 256 : half * 256 + 256],
                    start=(i == 0),
                    stop=(i == 7),
                )
            nc.vector.tensor_scalar_add(
                QT[p][:, qs], pp[:, 0, 0:256], bq_sb[:, p : p + 1]
            )

        ys_of = {}

        def out_sb(sb, oc):
            # self-contained: one oc = 4 accumulating matmuls + evac
            ss = slice(sb * 128, (sb + 1) * 128)
            if oc == 0:
                ys_of[sb] = oy.tile([128, 2, 512], F32R, name="ys")
            yp = ps_st.tile([128, 2, 512], F32, name="st")
            for p in range(4):
                nc.tensor.matmul(
                    yp[:, 0, :],
                    AT[p][:, ss],
                    wo[:, p, oc * 512 : (oc + 1) * 512],
                    start=(p == 0),
                    stop=(p == 3),
                )
            ys = ys_of[sb]
            nc.vector.tensor_copy(ys[:, oc, :], yp[:, 0, :])
            if oc == 1:
                del ys_of[sb]
                eng = (nc.gpsimd, nc.sync)[sb % 2]
                eng.dma_start(y_d[ss, :], ys[:])

        for u in range(16):
            p, qc = units[u]
            if p == 0 and qc < 3:
                q_load(qc + 1)
            # one block of non-score PE work per rep of 2 score pairs;
            # blocks are kept <= ~2us so the 2-deep st ring never lets
            # ACT run dry, and PE stays dense (HAM stays at K=8/8)
            blocks = []
            if u == 0:
                blocks += [lambda sb=sb: v_chunk(sb) for sb in range(8)]
                blocks += [lambda: q_chain(1, 0, 0), lambda: q_chain(1, 0, 1)]
            elif u == 1:
                blocks += [lambda sb=sb: v_chunk(sb) for sb in range(8, 16)]
                blocks += [
                    lambda: pv8(0, 0, 0),
                    lambda: pv8(0, 1, 0),
                    lambda: pv8(0, 0, 1),
                    lambda: (pv8(0, 1, 1), recip(0)),
                    lambda: q_chain(1, 1, 0),
                    lambda: q_chain(1, 1, 1),
                ]
            else:
                # ring-releasing DVE readers (bias-add, y-copy) must hit
                # the DVE FIFO before the reciprocals/norm multiplies
                if qc < 3:
                    blocks.append(lambda: q_chain(qc + 1, p, 0))
                    blocks.append(lambda: q_chain(qc + 1, p, 1))
                if u >= 6:
                    blocks.append(lambda: out_sb(u - 6, 0))
                    blocks.append(lambda: out_sb(u - 6, 1))
                blocks += [
                    lambda: pv8(u - 1, 0, 0),
                    lambda: pv8(u - 1, 1, 0),
                    lambda: pv8(u - 1, 0, 1),
                    lambda: (pv8(u - 1, 1, 1), recip(u - 1), norm(u - 2)),
                ]
            bi = 0
            for kb in range(0, 16, 2):
                sc_pair(u, kb)
                sc_pair(u, kb + 1)
                if bi < len(blocks):
                    blocks[bi]()
                    bi += 1
            while bi < len(blocks):
                blocks[bi]()
                bi += 1
        # tail: unit 15
        pv8(15, 0, 0)
        pv8(15, 1, 0)
        norm(14)
        pv8(15, 0, 1)
        pv8(15, 1, 1)
        recip(15)
        norm(15)
        for sb in range(10, 16):
            out_sb(sb, 0)
            out_sb(sb, 1)

        for cm in (oy_cm, bcs_cm, rec_cm, ech_cm, ps_pv_cm, ps_st_cm, stage_cm, warena_cm):
            cm.__exit__(None, None, None)

    nc.compile()
    return nc


def _get_nc():
    if "nc" not in _NC_CACHE:
        _NC_CACHE["nc"] = _build_nc()
    return _NC_CACHE["nc"]


def kernel(**inputs):
    import ml_dtypes
    from concourse import bass_utils

    BF = ml_dtypes.bfloat16
    q, k, v = inputs["query"], inputs["key"], inputs["value"]
    Wq, Wk, Wv, Wo = inputs["Wq"], inputs["Wk"], inputs["Wv"], inputs["Wo"]
    bq, bk, bv, bo = inputs["bq"], inputs["bk"], inputs["bv"], inputs["bo"]

    nc = _get_nc()
    in_maps = []
    for c in range(NCORES):
        b, hg = divmod(c, 2)
        r0 = hg * DG
        rs = slice(r0, r0 + DG)
        in_maps.append(
            {
                "xqT": np.ascontiguousarray(q[b].T.astype(BF)),
                "xkT": np.ascontiguousarray(k[b].T.astype(BF)),
                "xvT": np.ascontiguousarray(v[b].T.astype(BF)),
                "wqT": np.ascontiguousarray(Wq[rs, :].T.astype(BF)),
                "wkT": np.ascontiguousarray(Wk[rs, :].T.astype(BF)),
                "wvT": np.ascontiguousarray(Wv[rs, :].T.astype(BF)),
                "woT": np.ascontiguousarray(Wo[:, rs].T.astype(BF)),
                "bqc": np.ascontiguousarray(bq[rs].reshape(4, 128).T),
                "bkc": np.ascontiguousarray(bk[rs].reshape(4, 128).T),
                "bvr": np.ascontiguousarray(bv[rs]),
            }
        )
    import os

    trace = bool(os.environ.get("KERNEL_TRACE"))
    res = bass_utils.run_bass_kernel_spmd(
        nc, in_maps, core_ids=list(range(NCORES)), trace=trace
    )
    global LAST_EXEC_NS
    LAST_EXEC_NS = res.exec_time_ns
    out = np.empty((B, S, D), np.float32)
    for b in range(B):
        out[b] = res.results[2 * b]["y"] + res.results[2 * b + 1]["y"] + bo[None, :]
    return out
